# revision 10
# baseline (speedup 1.0000x reference)
"""Trainium2 Bass kernel for the dMaSIFConvBlock problem (v3).

Effective math (points/nuv/ranges are dead inputs in the reference):
    out = relu(features @ W1.T + b1) @ Wb.T + bb     (W1 = Wa@Wt fused on host)
a pointwise 16->16->16 MLP over 2M points.  Memory-bound; the 2e-2
correctness gate admits a bf16 stream, so the host stages features as
bf16 channel-major bundles (partition 16g+c = channel c of point-block
g) and upcasts the bf16 device output -- ~8 MB in + ~8 MB out per core,
no on-chip transposes, every DMA fully contiguous.

Device pipeline per core (61.04 superblocks of [128, 512]):
  - 128x128 bf16 stationary = 8x 16x16 weights along the diagonal; one
    N=512 matmul applies a layer to 4096 points.  Stationaries are
    switched once per two pairs (LDWEIGHTS halved vs per-pair).
  - PSUM: A = [128,1024] x2 bufs (layer-1 out), B = [128,1024] x2
    (layer-2 out).  FD=1024 drains are forced: 8 banks cannot double-
    buffer anything wider.
  - Layer-1 drain: ScalarE activation Relu+bias (PSUM f32 -> SBUF
    bf16).  Layer-2 drain: DVE tensor_scalar add-bias, except a couple
    assigned to ScalarE to balance the two 1x-rate PSUM readers.
  - v3 vs v2 (75.3us): slab schedule [1024, 2048, 4096x6, 2048, 1568]
    so the first matmul starts ~5us earlier and the tail chain is
    short; weight/bias consts ride 2 merged DMAs issued after the
    first slab's; final slab drains stream out per-pair.

Environment quirk handled at build time: this walrus build rejects
instructions with more than one semaphore wait; _split_multi_waits
moves every extra wait onto a standalone NoOp.
"""









import numpy as np
import ml_dtypes

import concourse.bass as bass
import concourse.tile as tile
from concourse import mybir
from concourse.bass_utils import run_bass_kernel_spmd

N_TOTAL = 2_000_000
C = 16
N_CORES = 8
N_SHARD = N_TOTAL // N_CORES      # 250_000 points per core

# 61.04 SBs per core; 1568 = 2*512 + 544 (tail pair padded to 544 cols)
SLAB_COLS = [1024, 2048] + [4096] * 6 + [2048, 1568]
SLABS = len(SLAB_COLS)
N_PAD = sum(8 * c for c in SLAB_COLS)          # 250_112 points
assert N_PAD >= N_SHARD

BF16 = mybir.dt.bfloat16
F32 = mybir.dt.float32


def _pairs(cols):
    out = []
    off = 0
    while off < cols:
        w = min(1024, cols - off)
        out.append((off, w))
        off += w
    return out


def _split_multi_waits(nc):
    for func in nc.m.functions:
        for bb in func.blocks:
            out = []
            changed = False
            for inst in bb.instructions:
                si = inst.sync_info
                if si is not None and len(si.on_wait) > 1:
                    waits = list(si.on_wait)
                    for j, w in enumerate(waits[:-1]):
                        out.append(
                            mybir.InstNoOp(
                                name=f"{inst.name}-xw{j}",
                                sync_info=mybir.SyncInfo(on_wait=[w], on_update=[]),
                                bass_nofuse=True,
                                engine=inst.engine,
                            )
                        )
                    si.on_wait = [waits[-1]]
                    inst.sync_info = si
                    changed = True
                out.append(inst)
            if changed:
                bb.instructions = out


def _build_program():
    nc = bass.Bass()
    n_el = N_PAD * C
    x_d = nc.dram_tensor("x", [n_el], BF16, kind="ExternalInput")
    y_d = nc.dram_tensor("y", [n_el], BF16, kind="ExternalOutput")
    w_d = nc.dram_tensor("wpack", [128, 256], BF16, kind="ExternalInput")
    b_d = nc.dram_tensor("bpack", [128, 2], F32, kind="ExternalInput")

    x_v, y_v = [], []
    base = 0
    for cols in SLAB_COLS:
        n = 128 * cols
        x_v.append(x_d.ap()[base : base + n].rearrange("(p m) -> p m", p=128))
        y_v.append(y_d.ap()[base : base + n].rearrange("(p m) -> p m", p=128))
        base += n
    relu = mybir.ActivationFunctionType.Relu
    ident = mybir.ActivationFunctionType.Identity

    with tile.TileContext(nc) as tc:
        with (
            tc.tile_pool(name="consts", bufs=1) as consts,
            tc.tile_pool(name="slabs", bufs=3) as slabs,
            tc.tile_pool(name="work", bufs=4) as work,
            tc.tile_pool(name="psum", bufs=2, space="PSUM") as psum,
        ):
            # slab 0 load first so compute starts as early as possible;
            # the two merged const DMAs slot in right behind it
            xs0 = slabs.tile([128, 4096], BF16, tag="xs")
            nc.sync.dma_start(xs0[:, : SLAB_COLS[0]], x_v[0])
            wpack = consts.tile([128, 256], BF16)
            nc.sync.dma_start(wpack[:], w_d.ap())
            bpack = consts.tile([128, 2], F32)
            nc.sync.dma_start(bpack[:], b_d.ap())
            bdw1 = wpack[:, 0:128]
            bdwb = wpack[:, 128:256]
            b1p = bpack[:, 0:1]
            b2p = bpack[:, 1:2]

            # While the first slab is in flight the PE sits idle and the
            # HAM clock gate stays cold (1.2 GHz) until ~3.4us of
            # sustained activity.  Burn that DMA-latency window on dummy
            # matmuls over a scratch tile so the real stream runs at
            # 2.4 GHz from its first instruction; a dummy activation
            # pulls the ~1.3us ACT table load off the critical path too.
            scratch = consts.tile([128, 512], BF16)
            nc.vector.memset(scratch[:], 0)
            wudum = work.tile([128, 1024], BF16, tag="h")
            nc.scalar.activation(
                wudum[:, :1], scratch[:, :1], mybir.ActivationFunctionType.Relu
            )
            wup = psum.tile([128, 1024], F32, tag="B")
            for _ in range(8):
                nc.tensor.matmul(wup[:, :512], scratch[:, :128], scratch[:])

            n_l2_on_scalar = 0
            pair_idx = 0
            for s in range(SLABS):
                cols = SLAB_COLS[s]
                if s == 0:
                    xs = xs0
                else:
                    xs = slabs.tile([128, 4096], BF16, tag="xs")
                    if cols > 2048:
                        hf = cols // 2
                        nc.sync.dma_start(xs[:, :hf], x_v[s][:, :hf])
                        nc.sync.dma_start(xs[:, hf:cols], x_v[s][:, hf:])
                    else:
                        nc.sync.dma_start(xs[:, :cols], x_v[s])
                ys = slabs.tile([128, 4096], BF16, tag="ys")
                pairs = _pairs(cols)
                last_slab = s == SLABS - 1
                # process pairs two at a time so each stationary is
                # loaded once per four matmuls
                for p0 in range(0, len(pairs), 2):
                    grp = pairs[p0 : p0 + 2]
                    aps, hs = [], []
                    for off, w in grp:
                        ap = psum.tile([128, 1024], F32, tag="A")
                        for k in range(0, w, 512):
                            kw = min(512, w - k)
                            nc.tensor.matmul(
                                ap[:, k : k + kw],
                                bdw1,
                                xs[:, off + k : off + k + kw],
                            )
                        aps.append(ap)
                    for (off, w), ap in zip(grp, aps):
                        h = work.tile([128, 1024], BF16, tag="h")
                        nc.scalar.activation(h[:, :w], ap[:, :w], relu, bias=b1p)
                        hs.append(h)
                    bps = []
                    for (off, w), h in zip(grp, hs):
                        bp = psum.tile([128, 1024], F32, tag="B")
                        for k in range(0, w, 512):
                            kw = min(512, w - k)
                            nc.tensor.matmul(
                                bp[:, k : k + kw], bdwb, h[:, k : k + kw]
                            )
                        bps.append(bp)
                    for (off, w), bp in zip(grp, bps):
                        # balance the two 1x-rate PSUM readers: ScalarE
                        # takes every 16th layer-2 drain
                        pair_idx += 1
                        if pair_idx % 16 == 0:
                            nc.scalar.activation(
                                ys[:, off : off + w], bp[:, :w], ident, bias=b2p
                            )
                            n_l2_on_scalar += 1
                        else:
                            nc.vector.tensor_scalar_add(
                                ys[:, off : off + w], bp[:, :w], b2p
                            )
                    if last_slab:
                        # per-pair, on the HWDGE ring (idle by now and
                        # quicker to first byte) to shorten the tail
                        for off, w in grp:
                            nc.sync.dma_start(
                                y_v[s][:, off : off + w], ys[:, off : off + w]
                            )
                    else:
                        o0 = grp[0][0]
                        o1 = grp[-1][0] + grp[-1][1]
                        nc.gpsimd.dma_start(y_v[s][:, o0:o1], ys[:, o0:o1])

    _split_multi_waits(nc)
    return nc


_NC = None


def _get_program():
    global _NC
    if _NC is None:
        _NC = _build_program()
    return _NC


def _f32_to_bf16_u16(x):
    u = np.ascontiguousarray(x, dtype=np.float32).view(np.uint32)
    rnd = ((u >> 16) & 1) + np.uint32(0x7FFF)
    return ((u + rnd) >> 16).astype(np.uint16)


def _prepare_in_maps(inputs):
    feats = np.asarray(inputs["features"], dtype=np.float32)
    Wt = np.asarray(inputs["Wt"], dtype=np.float32)
    bt = np.asarray(inputs["bt"], dtype=np.float32)
    Wa = np.asarray(inputs["Wa"], dtype=np.float32)
    ba = np.asarray(inputs["ba"], dtype=np.float32)
    Wb = np.asarray(inputs["Wb"], dtype=np.float32)
    bb = np.asarray(inputs["bb"], dtype=np.float32)

    W1 = (Wa @ Wt).astype(np.float32)
    b1 = (Wa @ bt + ba).astype(np.float32)

    wpack = np.zeros((128, 256), np.float32)
    for g in range(8):
        wpack[16 * g : 16 * g + 16, 16 * g : 16 * g + 16] = W1.T
        wpack[16 * g : 16 * g + 16, 128 + 16 * g : 128 + 16 * g + 16] = Wb.T
    wpack = _f32_to_bf16_u16(wpack).view(ml_dtypes.bfloat16)
    bpack = np.stack([np.tile(b1, 8), np.tile(bb, 8)], axis=1).astype(np.float32)

    fb = _f32_to_bf16_u16(feats).reshape(N_CORES, N_SHARD, C)
    padded = np.zeros((N_CORES, N_PAD, C), np.uint16)
    padded[:, :N_SHARD] = fb
    shards = np.empty((N_CORES, N_PAD * C), np.uint16)
    base_pt = 0
    base_el = 0
    for cols in SLAB_COLS:
        npts = 8 * cols
        chunk = padded[:, base_pt : base_pt + npts]
        chunk = chunk.reshape(N_CORES, 8, cols, C).transpose(0, 1, 3, 2)
        shards[:, base_el : base_el + npts * C] = chunk.reshape(N_CORES, npts * C)
        base_pt += npts
        base_el += npts * C

    shards_bf = shards.view(ml_dtypes.bfloat16)
    return [
        {"x": shards_bf[i], "wpack": wpack, "bpack": bpack}
        for i in range(N_CORES)
    ]


def _unstage(res):
    out = np.empty((N_TOTAL, C), np.float32)
    for i in range(N_CORES):
        y = np.asarray(res.results[i]["y"]).view(np.uint16)
        base_pt = 0
        base_el = 0
        dst = out[i * N_SHARD : (i + 1) * N_SHARD]
        for cols in SLAB_COLS:
            npts = 8 * cols
            blk = y[base_el : base_el + npts * C].reshape(8, C, cols)
            blk = blk.transpose(0, 2, 1).reshape(npts, C)
            lo = base_pt
            hi = min(base_pt + npts, N_SHARD)
            if lo < N_SHARD:
                u = blk[: hi - lo].astype(np.uint32) << 16
                dst[lo:hi] = u.view(np.float32)
            base_pt += npts
            base_el += npts * C
    return out


def _run(inputs, trace=False):
    nc = _get_program()
    in_maps = _prepare_in_maps(inputs)
    res = run_bass_kernel_spmd(nc, in_maps, core_ids=list(range(N_CORES)), trace=trace)
    out = _unstage(res)
    return out, res


def kernel(**inputs) -> np.ndarray:
    out, _ = _run(inputs, trace=False)
    return out


# revision 11
# speedup vs baseline: 1.0956x; 1.0956x over previous
"""Trainium2 Bass kernel for the dMaSIFConvBlock problem (v3).

Effective math (points/nuv/ranges are dead inputs in the reference):
    out = relu(features @ W1.T + b1) @ Wb.T + bb     (W1 = Wa@Wt fused on host)
a pointwise 16->16->16 MLP over 2M points.  Memory-bound; the 2e-2
correctness gate admits a bf16 stream, so the host stages features as
bf16 channel-major bundles (partition 16g+c = channel c of point-block
g) and upcasts the bf16 device output -- ~8 MB in + ~8 MB out per core,
no on-chip transposes, every DMA fully contiguous.

Device pipeline per core (61.04 superblocks of [128, 512]):
  - 128x128 bf16 stationary = 8x 16x16 weights along the diagonal; one
    N=512 matmul applies a layer to 4096 points.  Stationaries are
    switched once per two pairs (LDWEIGHTS halved vs per-pair).
  - PSUM: A = [128,1024] x2 bufs (layer-1 out), B = [128,1024] x2
    (layer-2 out).  FD=1024 drains are forced: 8 banks cannot double-
    buffer anything wider.
  - Layer-1 drain: ScalarE activation Relu+bias (PSUM f32 -> SBUF
    bf16).  Layer-2 drain: DVE tensor_scalar add-bias, except a couple
    assigned to ScalarE to balance the two 1x-rate PSUM readers.
  - v3 vs v2 (75.3us): slab schedule [1024, 2048, 4096x6, 2048, 1568]
    so the first matmul starts ~5us earlier and the tail chain is
    short; weight/bias consts ride 2 merged DMAs issued after the
    first slab's; final slab drains stream out per-pair.

Environment quirk handled at build time: this walrus build rejects
instructions with more than one semaphore wait; _split_multi_waits
moves every extra wait onto a standalone NoOp.
"""







import numpy as np
import ml_dtypes

import concourse.bass as bass
import concourse.tile as tile
from concourse import mybir
from concourse.bass_utils import run_bass_kernel_spmd

N_TOTAL = 2_000_000
C = 16
N_CORES = 8
N_SHARD = N_TOTAL // N_CORES      # 250_000 points per core

# 61.04 SBs per core; 1568 = 2*512 + 544 (tail pair padded to 544 cols)
SLAB_COLS = [1024, 2048] + [4096] * 6 + [2048, 1568]
SLABS = len(SLAB_COLS)
N_PAD = sum(8 * c for c in SLAB_COLS)          # 250_112 points
assert N_PAD >= N_SHARD

BF16 = mybir.dt.bfloat16
F32 = mybir.dt.float32


def _pairs(cols):
    out = []
    off = 0
    while off < cols:
        w = min(1024, cols - off)
        out.append((off, w))
        off += w
    return out


def _split_multi_waits(nc):
    for func in nc.m.functions:
        for bb in func.blocks:
            out = []
            changed = False
            for inst in bb.instructions:
                si = inst.sync_info
                if si is not None and len(si.on_wait) > 1:
                    waits = list(si.on_wait)
                    for j, w in enumerate(waits[:-1]):
                        out.append(
                            mybir.InstNoOp(
                                name=f"{inst.name}-xw{j}",
                                sync_info=mybir.SyncInfo(on_wait=[w], on_update=[]),
                                bass_nofuse=True,
                                engine=inst.engine,
                            )
                        )
                    si.on_wait = [waits[-1]]
                    inst.sync_info = si
                    changed = True
                out.append(inst)
            if changed:
                bb.instructions = out


def _build_program():
    nc = bass.Bass()
    n_el = N_PAD * C
    x_d = nc.dram_tensor("x", [n_el], BF16, kind="ExternalInput")
    y_d = nc.dram_tensor("y", [n_el], BF16, kind="ExternalOutput")
    w_d = nc.dram_tensor("wpack", [128, 256], BF16, kind="ExternalInput")
    b_d = nc.dram_tensor("bpack", [128, 2], F32, kind="ExternalInput")

    x_v, y_v = [], []
    base = 0
    for cols in SLAB_COLS:
        n = 128 * cols
        x_v.append(x_d.ap()[base : base + n].rearrange("(p m) -> p m", p=128))
        y_v.append(y_d.ap()[base : base + n].rearrange("(p m) -> p m", p=128))
        base += n
    relu = mybir.ActivationFunctionType.Relu
    ident = mybir.ActivationFunctionType.Identity

    with tile.TileContext(nc) as tc:
        with (
            tc.tile_pool(name="consts", bufs=1) as consts,
            tc.tile_pool(name="slabs", bufs=3) as slabs,
            tc.tile_pool(name="work", bufs=4) as work,
            tc.tile_pool(name="psum", bufs=2, space="PSUM") as psum,
        ):
            # slab 0 load first so compute starts as early as possible;
            # the two merged const DMAs slot in right behind it
            xs0 = slabs.tile([128, 4096], BF16, tag="xs")
            nc.sync.dma_start(xs0[:, : SLAB_COLS[0]], x_v[0])
            wpack = consts.tile([128, 256], BF16)
            nc.sync.dma_start(wpack[:], w_d.ap())
            bpack = consts.tile([128, 2], F32)
            nc.sync.dma_start(bpack[:], b_d.ap())
            bdw1 = wpack[:, 0:128]
            bdwb = wpack[:, 128:256]
            b1p = bpack[:, 0:1]
            b2p = bpack[:, 1:2]

            # While the first slab is in flight the PE sits idle and the
            # HAM clock gate stays cold (1.2 GHz) until ~3.4us of
            # sustained activity.  Burn that DMA-latency window on dummy
            # matmuls over a scratch tile so the real stream runs at
            # 2.4 GHz from its first instruction; a dummy activation
            # pulls the ~1.3us ACT table load off the critical path too.
            scratch = consts.tile([128, 512], BF16)
            nc.vector.memset(scratch[:], 0)
            wudum = work.tile([128, 1024], BF16, tag="h")
            nc.scalar.activation(
                wudum[:, :1], scratch[:, :1], mybir.ActivationFunctionType.Relu
            )
            wup = psum.tile([128, 1024], F32, tag="B")
            for _ in range(8):
                nc.tensor.matmul(wup[:, :512], scratch[:, :128], scratch[:])

            n_l2_on_scalar = 0
            pair_idx = 0
            for s in range(SLABS):
                cols = SLAB_COLS[s]
                if s == 0:
                    xs = xs0
                else:
                    xs = slabs.tile([128, 4096], BF16, tag="xs")
                    if cols > 2048:
                        hf = cols // 2
                        nc.sync.dma_start(xs[:, :hf], x_v[s][:, :hf])
                        nc.sync.dma_start(xs[:, hf:cols], x_v[s][:, hf:])
                    else:
                        nc.sync.dma_start(xs[:, :cols], x_v[s])
                ys = slabs.tile([128, 4096], BF16, tag="ys")
                pairs = _pairs(cols)
                last_slab = s == SLABS - 1
                # process pairs two at a time so each stationary is
                # loaded once per four matmuls
                for p0 in range(0, len(pairs), 2):
                    grp = pairs[p0 : p0 + 2]
                    aps, hs = [], []
                    for off, w in grp:
                        ap = psum.tile([128, 1024], F32, tag="A")
                        for k in range(0, w, 512):
                            kw = min(512, w - k)
                            nc.tensor.matmul(
                                ap[:, k : k + kw],
                                bdw1,
                                xs[:, off + k : off + k + kw],
                            )
                        aps.append(ap)
                    for (off, w), ap in zip(grp, aps):
                        h = work.tile([128, 1024], BF16, tag="h")
                        nc.scalar.activation(h[:, :w], ap[:, :w], relu, bias=b1p)
                        hs.append(h)
                    bps = []
                    for (off, w), h in zip(grp, hs):
                        bp = psum.tile([128, 1024], F32, tag="B")
                        for k in range(0, w, 512):
                            kw = min(512, w - k)
                            nc.tensor.matmul(
                                bp[:, k : k + kw], bdwb, h[:, k : k + kw]
                            )
                        bps.append(bp)
                    for (off, w), bp in zip(grp, bps):
                        # balance the two 1x-rate PSUM readers: ScalarE
                        # takes every 16th layer-2 drain
                        pair_idx += 1
                        if pair_idx % 16 == 0:
                            nc.scalar.activation(
                                ys[:, off : off + w], bp[:, :w], ident, bias=b2p
                            )
                            n_l2_on_scalar += 1
                        else:
                            nc.vector.tensor_scalar_add(
                                ys[:, off : off + w], bp[:, :w], b2p
                            )
                    if last_slab:
                        # per-pair, on the HWDGE ring (idle by now and
                        # quicker to first byte) to shorten the tail
                        for off, w in grp:
                            nc.sync.dma_start(
                                y_v[s][:, off : off + w], ys[:, off : off + w]
                            )
                    else:
                        o0 = grp[0][0]
                        o1 = grp[-1][0] + grp[-1][1]
                        nc.gpsimd.dma_start(y_v[s][:, o0:o1], ys[:, o0:o1])

    _split_multi_waits(nc)
    return nc


_NC = None


def _get_program():
    global _NC
    if _NC is None:
        _NC = _build_program()
    return _NC


def _f32_to_bf16_u16(x):
    u = np.ascontiguousarray(x, dtype=np.float32).view(np.uint32)
    rnd = ((u >> 16) & 1) + np.uint32(0x7FFF)
    return ((u + rnd) >> 16).astype(np.uint16)


def _prepare_in_maps(inputs):
    feats = np.asarray(inputs["features"], dtype=np.float32)
    Wt = np.asarray(inputs["Wt"], dtype=np.float32)
    bt = np.asarray(inputs["bt"], dtype=np.float32)
    Wa = np.asarray(inputs["Wa"], dtype=np.float32)
    ba = np.asarray(inputs["ba"], dtype=np.float32)
    Wb = np.asarray(inputs["Wb"], dtype=np.float32)
    bb = np.asarray(inputs["bb"], dtype=np.float32)

    W1 = (Wa @ Wt).astype(np.float32)
    b1 = (Wa @ bt + ba).astype(np.float32)

    wpack = np.zeros((128, 256), np.float32)
    for g in range(8):
        wpack[16 * g : 16 * g + 16, 16 * g : 16 * g + 16] = W1.T
        wpack[16 * g : 16 * g + 16, 128 + 16 * g : 128 + 16 * g + 16] = Wb.T
    wpack = _f32_to_bf16_u16(wpack).view(ml_dtypes.bfloat16)
    bpack = np.stack([np.tile(b1, 8), np.tile(bb, 8)], axis=1).astype(np.float32)

    fb = _f32_to_bf16_u16(feats).reshape(N_CORES, N_SHARD, C)
    padded = np.zeros((N_CORES, N_PAD, C), np.uint16)
    padded[:, :N_SHARD] = fb
    shards = np.empty((N_CORES, N_PAD * C), np.uint16)
    base_pt = 0
    base_el = 0
    for cols in SLAB_COLS:
        npts = 8 * cols
        chunk = padded[:, base_pt : base_pt + npts]
        chunk = chunk.reshape(N_CORES, 8, cols, C).transpose(0, 1, 3, 2)
        shards[:, base_el : base_el + npts * C] = chunk.reshape(N_CORES, npts * C)
        base_pt += npts
        base_el += npts * C

    shards_bf = shards.view(ml_dtypes.bfloat16)
    return [
        {"x": shards_bf[i], "wpack": wpack, "bpack": bpack}
        for i in range(N_CORES)
    ]


def _unstage(res):
    out = np.empty((N_TOTAL, C), np.float32)
    for i in range(N_CORES):
        y = np.asarray(res.results[i]["y"]).view(np.uint16)
        base_pt = 0
        base_el = 0
        dst = out[i * N_SHARD : (i + 1) * N_SHARD]
        for cols in SLAB_COLS:
            npts = 8 * cols
            blk = y[base_el : base_el + npts * C].reshape(8, C, cols)
            blk = blk.transpose(0, 2, 1).reshape(npts, C)
            lo = base_pt
            hi = min(base_pt + npts, N_SHARD)
            if lo < N_SHARD:
                u = blk[: hi - lo].astype(np.uint32) << 16
                dst[lo:hi] = u.view(np.float32)
            base_pt += npts
            base_el += npts * C
    return out


def _run(inputs, trace=False):
    nc = _get_program()
    in_maps = _prepare_in_maps(inputs)
    res = run_bass_kernel_spmd(nc, in_maps, core_ids=list(range(N_CORES)), trace=trace)
    out = _unstage(res)
    return out, res


def kernel(**inputs) -> np.ndarray:
    out, _ = _run(inputs, trace=False)
    return out
